# revision 36
# baseline (speedup 1.0000x reference)
"""CrossTuckerLayer kernel for 8x Trainium2 NeuronCores (Bass/Tile).

Computes y = einsum('bnvade,ABCDEF,oA,pB,qC,aD,dE,eF->bnvopq', ...)
reshaped to [b, n, v, o*p, q], data-parallel over the 2048 (b,n,v) samples
(256 per core). All HBM I/O is bf16 (harness gate is rel_err < 2e-2; this
path lands ~3.4e-3), halving DMA traffic vs fp32.

Host folds the tiny Tucker factors (all <10K params) into two matrices:
  M    [16384, 8] = einsum('ABCDEF,aD,dE,eF->adeABC', core, a0, a1, a2)
  Wout [8, 32768] = einsum('oA,pB,qC->ABCopq', u0, u1, u2)

Per core the 256 samples split into two 128-sample windows:
  stage A (PE): s2_w[8, 128] = sum over 128 fin-chunks of
      M_ck[128f, 8]^T @ x_ck[128f, 128s]; M is the stationary operand so
      the result lands directly in the [8, s] layout stage C needs.
      Both windows run back-to-back, chasing the x DMA stream (the PE is
      nowhere near the bottleneck, so A costs no extra wall-clock).
  s2 is then replicated to partition blocks 0/32/64/96 (one DVE copy +
      three SBUF->SBUF DMAs) so stage C can row-tile the PE.
  stage C (PE): y[128s, 512] tiles = s2_w[8, 128]^T @ Wout[8, 512] with
      K=8 only — so four matmuls run CONCURRENTLY in distinct 32-row
      PE groups via tile_position=(32i, 0) (Wout is staged per row-group
      host-side). Stage C runs uninterrupted — no stage-A matmuls in the
      PE FIFO ahead of C rounds — so its cadence is set purely by the
      psum->sbuf copies (vector/scalar alternating, the only two engines
      that can read PSUM on TRN2) and the y DMA rate. Half-stage y
      stores alternate between the two HWDGE queues so neither queue
      idles across a stage boundary.

DMA issue is staggered via tile-pool reuse (xp bufs): the DMA engines
round-robin across all outstanding transfers on a queue, so issuing
everything upfront makes the FIRST tile complete last.
"""

import numpy as np
import ml_dtypes

import concourse.bass as bass
import concourse.bacc as bacc
import concourse.mybir as mybir
from concourse.tile import TileContext
from concourse.bass_utils import run_bass_kernel_spmd

F32 = mybir.dt.float32
BF16 = mybir.dt.bfloat16
BF = ml_dtypes.bfloat16

NCORES = 8
S_TOT = 2048          # 4*64*8 samples
S = S_TOT // NCORES   # 256 per core
FIN = 16 * 16 * 64    # 16384
FOUT = 256 * 128      # 32768
NCK = FIN // 128      # 128 contraction chunks of 128
WIN = 128             # samples per window
N_WIN = S // WIN      # 2
G_CK_W = (32, 16)     # chunks per x DMA tile: big tiles first for
                      # DRAM read locality, smaller for window 1 so the
                      # final stage-A group (the x->C transition tail)
                      # is short
N_G_W = tuple(NCK // g for g in G_CK_W)
YCHUNK = 512          # one matmul's psum cols (fits a 2KB fp32 bank)
YSTAGE = 4096         # cols per y staging tile / output DMA
N_YSTAGE = FOUT // YSTAGE  # 8 per window
NTILE = 4             # concurrent row-group matmuls in stage C
NSLOT = FOUT // YCHUNK // NTILE  # 16 column slots per row-group


def _host_weights(core, u0, u1, u2, a0, a1, a2):
    """Fold the Tucker factors into M [128f, 128ck*8] and the row-group
    staged Wout [128, NSLOT*512]."""
    M = np.einsum(
        "ABCDEF,aD,dE,eF->adeABC",
        core.astype(np.float64), a0.astype(np.float64),
        a1.astype(np.float64), a2.astype(np.float64),
    ).reshape(FIN, 8)
    # SBUF layout [f, ck*8 + r] where fin = ck*128 + f
    Mdev = np.ascontiguousarray(
        M.reshape(NCK, 128, 8).transpose(1, 0, 2).reshape(128, NCK * 8)
    ).astype(BF)

    Wout = np.einsum(
        "oA,pB,qC->ABCopq",
        u0.astype(np.float64), u1.astype(np.float64), u2.astype(np.float64),
    ).reshape(8, FOUT)
    # chunk c of 512 cols -> row-group i = c % 4, col slot j = c // 4;
    # staged at SBUF partitions 32i..32i+8 (rows 8..31 of each group are
    # padding, never read).
    wl4 = np.zeros((128, NSLOT * YCHUNK), dtype=np.float64)
    for c in range(FOUT // YCHUNK):
        i, j = c % NTILE, c // NTILE
        wl4[32 * i:32 * i + 8, j * YCHUNK:(j + 1) * YCHUNK] = \
            Wout[:, c * YCHUNK:(c + 1) * YCHUNK]
    return Mdev, np.ascontiguousarray(wl4.astype(BF))


def _host_x(x):
    """x [2048, FIN] f32 -> per-core dev layout [128f, w*16K + ck*128 + s]."""
    xb = x.reshape(S_TOT, FIN).astype(BF)
    xd = np.ascontiguousarray(
        xb.reshape(NCORES, N_WIN, WIN, NCK, 128).transpose(0, 4, 1, 3, 2)
    ).reshape(NCORES, 128, N_WIN * FIN)
    return xd


def _build():
    nc = bacc.Bacc("TRN2", target_bir_lowering=False, debug=False)
    x_d = nc.dram_tensor("x", [128, N_WIN * FIN], BF16, kind="ExternalInput")
    m_d = nc.dram_tensor("m", [128, NCK * 8], BF16, kind="ExternalInput")
    wl_d = nc.dram_tensor("wl", [128, NSLOT * YCHUNK], BF16,
                          kind="ExternalInput")
    y_d = nc.dram_tensor("y", [S, FOUT], BF16, kind="ExternalOutput")

    with TileContext(nc) as tc:
        with (
            tc.tile_pool(name="consts", bufs=1) as cpool,
            tc.tile_pool(name="xp", bufs=4) as xp,
            tc.tile_pool(name="s2p", bufs=2) as s2p,
            tc.tile_pool(name="yp", bufs=6) as yp,
            tc.tile_pool(name="psA", bufs=1, space=bass.MemorySpace.PSUM) as psA,
            tc.tile_pool(name="psC", bufs=7, space=bass.MemorySpace.PSUM) as psC,
        ):
            mm = cpool.tile([128, NCK * 8], BF16)
            nc.sync.dma_start(mm[:], m_d[:])
            wl = cpool.tile([128, NSLOT * YCHUNK], BF16)
            nc.scalar.dma_start(wl[:], wl_d[:])

            # x DMAs: issue order == consumption order; xp bufs throttle
            # issue so in-flight transfers stay few (round-robin engines
            # otherwise finish everything at once, starving stage A).
            x_tiles = {}
            i = 0
            for w in range(N_WIN):
                gck = G_CK_W[w]
                for g in range(N_G_W[w]):
                    xg = xp.tile([128, max(G_CK_W) * WIN], BF16, tag="xg",
                                 name=f"x_{w}_{g}")
                    eng = nc.sync if i % 2 == 0 else nc.scalar
                    eng.dma_start(
                        xg[:, :gck * WIN],
                        x_d[:, (w * NCK + g * gck) * WIN:
                               (w * NCK + (g + 1) * gck) * WIN],
                    )
                    x_tiles[(w, g)] = xg
                    i += 1

            sA = [psA.tile([8, WIN], F32, tag="sA", name=f"sA_{w}")
                  for w in range(N_WIN)]
            s2r = [s2p.tile([128, WIN], BF16, tag="s2", name=f"s2_{w}")
                   for w in range(N_WIN)]

            def emit_a_group(w, g):
                for ckl in range(G_CK_W[w]):
                    ck = g * G_CK_W[w] + ckl
                    nc.tensor.matmul(
                        sA[w][:],
                        mm[:, ck * 8:(ck + 1) * 8],
                        x_tiles[(w, g)][:, ckl * WIN:(ckl + 1) * WIN],
                        start=(ck == 0), stop=(ck == NCK - 1),
                        skip_group_check=True,
                    )

            def emit_s2_replicate(w):
                # bf16 downcast into row-group 0, then fan out to 32/64/96
                nc.vector.tensor_copy(s2r[w][0:8, :], sA[w][:])
                for i in range(1, NTILE):
                    nc.sync.dma_start(s2r[w][32 * i:32 * i + 8, :],
                                      s2r[w][0:8, :])

            def emit_c_stage(w, st):
                y_sb = yp.tile([128, YSTAGE], BF16, tag="ysb", name="y_sb")
                for h in range(2):
                    slot = st * 2 + h
                    # 4 concurrent row-group matmuls, one psum bank each
                    pss = []
                    for i in range(NTILE):
                        y_ps = psC.tile([128, YCHUNK], F32, tag="yps",
                                        name="y_ps")
                        nc.tensor.matmul(
                            y_ps[:],
                            s2r[w][32 * i:32 * i + 8, :],
                            wl[32 * i:32 * i + 8,
                               slot * YCHUNK:(slot + 1) * YCHUNK],
                            start=True, stop=True,
                            tile_position=(32 * i, 0),
                        )
                        pss.append(y_ps)
                    # one engine owns this half-stage: the two engines
                    # run concurrent halves, and the scalar engine issues
                    # its own half's store with no cross-engine sem hop
                    for i in range(NTILE):
                        c8 = h * NTILE + i
                        dst = y_sb[:, c8 * YCHUNK:(c8 + 1) * YCHUNK]
                        if h == 0:
                            nc.vector.tensor_copy(dst, pss[i][:])
                        else:
                            nc.scalar.copy(dst, pss[i][:])
                    eng = nc.sync if h == 0 else nc.scalar
                    eng.dma_start(
                        y_d[w * WIN:(w + 1) * WIN,
                            st * YSTAGE + h * 4 * YCHUNK:
                            st * YSTAGE + (h + 1) * 4 * YCHUNK],
                        y_sb[:, h * 4 * YCHUNK:(h + 1) * 4 * YCHUNK],
                    )

            # stage A for both windows, chasing the x DMA stream; then
            # stage C uninterrupted so copies/y-DMA pace the pipeline
            # with no stage-A matmuls in the PE FIFO ahead of C rounds.
            for w in range(N_WIN):
                for g in range(N_G_W[w]):
                    emit_a_group(w, g)
                emit_s2_replicate(w)
            for w in range(N_WIN):
                for st in range(N_YSTAGE):
                    emit_c_stage(w, st)
    nc.compile()
    return nc


_NC_CACHE = []


def _get_nc():
    if not _NC_CACHE:
        _NC_CACHE.append(_build())
    return _NC_CACHE[0]


def run(inputs, trace=False):
    x = np.asarray(inputs["x"], dtype=np.float32)
    Mdev, wl4 = _host_weights(
        np.asarray(inputs["core"]),
        np.asarray(inputs["u0"]), np.asarray(inputs["u1"]),
        np.asarray(inputs["u2"]),
        np.asarray(inputs["a0"]), np.asarray(inputs["a1"]),
        np.asarray(inputs["a2"]),
    )
    xd = _host_x(x)
    nc = _get_nc()
    in_maps = []
    for i in range(NCORES):
        in_maps.append({
            "x": xd[i],
            "m": Mdev,
            "wl": wl4,
        })
    res = run_bass_kernel_spmd(
        nc, in_maps, core_ids=list(range(NCORES)), trace=trace,
    )
    y = np.concatenate([np.asarray(r["y"]) for r in res.results], axis=0)
    y = y.astype(np.float32).reshape(4, 64, 8, 256, 128)
    return y, res


def kernel(**inputs) -> np.ndarray:
    y, _ = run(inputs, trace=False)
    return y


# revision 38
# speedup vs baseline: 1.0128x; 1.0128x over previous
"""CrossTuckerLayer kernel for 8x Trainium2 NeuronCores (Bass/Tile).

Computes y = einsum('bnvade,ABCDEF,oA,pB,qC,aD,dE,eF->bnvopq', ...)
reshaped to [b, n, v, o*p, q], data-parallel over the 2048 (b,n,v) samples
(256 per core). All HBM I/O is bf16 (harness gate is rel_err < 2e-2; this
path lands ~3.4e-3), halving DMA traffic vs fp32.

Host folds the tiny Tucker factors (all <10K params) into two matrices:
  M    [16384, 8] = einsum('ABCDEF,aD,dE,eF->adeABC', core, a0, a1, a2)
  Wout [8, 32768] = einsum('oA,pB,qC->ABCopq', u0, u1, u2)

Per core the 256 samples split into two 128-sample windows:
  stage A (PE): s2_w[8, 128] = sum over 128 fin-chunks of
      M_ck[128f, 8]^T @ x_ck[128f, 128s]; M is the stationary operand so
      the result lands directly in the [8, s] layout stage C needs.
      Both windows run back-to-back, chasing the x DMA stream (the PE is
      nowhere near the bottleneck, so A costs no extra wall-clock).
  s2 is then replicated to partition blocks 0/32/64/96 (one DVE copy +
      three SBUF->SBUF DMAs) so stage C can row-tile the PE.
  stage C (PE): y[128s, 512] tiles = s2_w[8, 128]^T @ Wout[8, 512] with
      K=8 only — so four matmuls run CONCURRENTLY in distinct 32-row
      PE groups via tile_position=(32i, 0) (Wout is staged per row-group
      host-side). Stage C runs uninterrupted — no stage-A matmuls in the
      PE FIFO ahead of C rounds — so its cadence is set purely by the
      psum->sbuf copies (vector/scalar alternating, the only two engines
      that can read PSUM on TRN2) and the y DMA rate. Half-stage y
      stores alternate between the two HWDGE queues so neither queue
      idles across a stage boundary.

DMA issue is staggered via tile-pool reuse (xp bufs): the DMA engines
round-robin across all outstanding transfers on a queue, so issuing
everything upfront makes the FIRST tile complete last.
"""

import numpy as np
import ml_dtypes

import concourse.bass as bass
import concourse.bacc as bacc
import concourse.mybir as mybir
from concourse.tile import TileContext
from concourse.bass_utils import run_bass_kernel_spmd

F32 = mybir.dt.float32
BF16 = mybir.dt.bfloat16
BF = ml_dtypes.bfloat16

NCORES = 8
S_TOT = 2048          # 4*64*8 samples
S = S_TOT // NCORES   # 256 per core
FIN = 16 * 16 * 64    # 16384
FOUT = 256 * 128      # 32768
NCK = FIN // 128      # 128 contraction chunks of 128
WIN = 128             # samples per window
N_WIN = S // WIN      # 2
G_CK = 32             # chunks per x DMA tile: 1MB transfers with 8KB
                      # contiguous per partition read ~355GB/s vs ~285
                      # for 0.5MB tiles (DRAM read locality)
N_G = NCK // G_CK     # 4 x tiles per window
YCHUNK = 512          # one matmul's psum cols (fits a 2KB fp32 bank)
YSTAGE = 4096         # cols per y staging tile / output DMA
N_YSTAGE = FOUT // YSTAGE  # 8 per window
NTILE = 4             # concurrent row-group matmuls in stage C
NSLOT = FOUT // YCHUNK // NTILE  # 16 column slots per row-group


def _host_weights(core, u0, u1, u2, a0, a1, a2):
    """Fold the Tucker factors into M [128f, 128ck*8] and the row-group
    staged Wout [128, NSLOT*512]."""
    M = np.einsum(
        "ABCDEF,aD,dE,eF->adeABC",
        core.astype(np.float64), a0.astype(np.float64),
        a1.astype(np.float64), a2.astype(np.float64),
    ).reshape(FIN, 8)
    # SBUF layout [f, ck*8 + r] where fin = ck*128 + f
    Mdev = np.ascontiguousarray(
        M.reshape(NCK, 128, 8).transpose(1, 0, 2).reshape(128, NCK * 8)
    ).astype(BF)

    Wout = np.einsum(
        "oA,pB,qC->ABCopq",
        u0.astype(np.float64), u1.astype(np.float64), u2.astype(np.float64),
    ).reshape(8, FOUT)
    # chunk c of 512 cols -> row-group i = c % 4, col slot j = c // 4;
    # staged at SBUF partitions 32i..32i+8 (rows 8..31 of each group are
    # padding, never read).
    wl4 = np.zeros((128, NSLOT * YCHUNK), dtype=np.float64)
    for c in range(FOUT // YCHUNK):
        i, j = c % NTILE, c // NTILE
        wl4[32 * i:32 * i + 8, j * YCHUNK:(j + 1) * YCHUNK] = \
            Wout[:, c * YCHUNK:(c + 1) * YCHUNK]
    return Mdev, np.ascontiguousarray(wl4.astype(BF))


def _host_x(x):
    """x [2048, FIN] f32 -> per-core dev layout [128f, w*16K + ck*128 + s]."""
    xb = x.reshape(S_TOT, FIN).astype(BF)
    xd = np.ascontiguousarray(
        xb.reshape(NCORES, N_WIN, WIN, NCK, 128).transpose(0, 4, 1, 3, 2)
    ).reshape(NCORES, 128, N_WIN * FIN)
    return xd


def _build():
    nc = bacc.Bacc("TRN2", target_bir_lowering=False, debug=False)
    x_d = nc.dram_tensor("x", [128, N_WIN * FIN], BF16, kind="ExternalInput")
    m_d = nc.dram_tensor("m", [128, NCK * 8], BF16, kind="ExternalInput")
    wl_d = nc.dram_tensor("wl", [128, NSLOT * YCHUNK], BF16,
                          kind="ExternalInput")
    y_d = nc.dram_tensor("y", [S, FOUT], BF16, kind="ExternalOutput")

    with TileContext(nc) as tc:
        with (
            tc.tile_pool(name="consts", bufs=1) as cpool,
            tc.tile_pool(name="xp", bufs=4) as xp,
            tc.tile_pool(name="s2p", bufs=2) as s2p,
            tc.tile_pool(name="yp", bufs=6) as yp,
            tc.tile_pool(name="psA", bufs=1, space=bass.MemorySpace.PSUM) as psA,
            tc.tile_pool(name="psC", bufs=7, space=bass.MemorySpace.PSUM) as psC,
        ):
            mm = cpool.tile([128, NCK * 8], BF16)
            nc.sync.dma_start(mm[:], m_d[:])
            wl = cpool.tile([128, NSLOT * YCHUNK], BF16)
            nc.scalar.dma_start(wl[:], wl_d[:])

            # x DMAs: issue order == consumption order; xp bufs throttle
            # issue so in-flight transfers stay few (round-robin engines
            # otherwise finish everything at once, starving stage A).
            x_tiles = {}
            for w in range(N_WIN):
                for g in range(N_G):
                    i = w * N_G + g
                    xg = xp.tile([128, G_CK * WIN], BF16, tag="xg",
                                 name=f"x_{w}_{g}")
                    eng = nc.sync if i % 2 == 0 else nc.scalar
                    eng.dma_start(
                        xg[:],
                        x_d[:, (w * NCK + g * G_CK) * WIN:
                               (w * NCK + (g + 1) * G_CK) * WIN],
                    )
                    x_tiles[(w, g)] = xg

            sA = [psA.tile([8, WIN], F32, tag="sA", name=f"sA_{w}")
                  for w in range(N_WIN)]
            s2r = [s2p.tile([128, WIN], BF16, tag="s2", name=f"s2_{w}")
                   for w in range(N_WIN)]

            def emit_a_group(w, g):
                for ckl in range(G_CK):
                    ck = g * G_CK + ckl
                    nc.tensor.matmul(
                        sA[w][:],
                        mm[:, ck * 8:(ck + 1) * 8],
                        x_tiles[(w, g)][:, ckl * WIN:(ckl + 1) * WIN],
                        start=(ck == 0), stop=(ck == NCK - 1),
                        skip_group_check=True,
                    )

            def emit_s2_replicate(w):
                # bf16 downcast into row-group 0, then fan out to 32/64/96
                nc.vector.tensor_copy(s2r[w][0:8, :], sA[w][:])
                for i in range(1, NTILE):
                    nc.sync.dma_start(s2r[w][32 * i:32 * i + 8, :],
                                      s2r[w][0:8, :])

            def emit_c_stage(w, st):
                y_sb = yp.tile([128, YSTAGE], BF16, tag="ysb", name="y_sb")
                for h in range(2):
                    slot = st * 2 + h
                    # 4 concurrent row-group matmuls, one psum bank each
                    pss = []
                    for i in range(NTILE):
                        y_ps = psC.tile([128, YCHUNK], F32, tag="yps",
                                        name="y_ps")
                        nc.tensor.matmul(
                            y_ps[:],
                            s2r[w][32 * i:32 * i + 8, :],
                            wl[32 * i:32 * i + 8,
                               slot * YCHUNK:(slot + 1) * YCHUNK],
                            start=True, stop=True,
                            tile_position=(32 * i, 0),
                        )
                        pss.append(y_ps)
                    # one engine owns this half-stage: the two engines
                    # run concurrent halves, and the scalar engine issues
                    # its own half's store with no cross-engine sem hop
                    for i in range(NTILE):
                        c8 = h * NTILE + i
                        dst = y_sb[:, c8 * YCHUNK:(c8 + 1) * YCHUNK]
                        if h == 0:
                            nc.vector.tensor_copy(dst, pss[i][:])
                        else:
                            nc.scalar.copy(dst, pss[i][:])
                    eng = nc.sync if h == 0 else nc.scalar
                    eng.dma_start(
                        y_d[w * WIN:(w + 1) * WIN,
                            st * YSTAGE + h * 4 * YCHUNK:
                            st * YSTAGE + (h + 1) * 4 * YCHUNK],
                        y_sb[:, h * 4 * YCHUNK:(h + 1) * 4 * YCHUNK],
                    )

            # stage A for both windows, chasing the x DMA stream. Two
            # early C0 stages slide in before A1's final group so their
            # y stores fill the DMA idle while the last x tile lands and
            # the window-1 replication chain drains; the rest of stage C
            # runs uninterrupted.
            for g in range(N_G):
                emit_a_group(0, g)
            emit_s2_replicate(0)
            for g in range(N_G - 1):
                emit_a_group(1, g)
            emit_c_stage(0, 0)
            emit_c_stage(0, 1)
            emit_a_group(1, N_G - 1)
            emit_s2_replicate(1)
            for st in range(2, N_YSTAGE):
                emit_c_stage(0, st)
            for st in range(N_YSTAGE):
                emit_c_stage(1, st)
    nc.compile()
    return nc


_NC_CACHE = []


def _get_nc():
    if not _NC_CACHE:
        _NC_CACHE.append(_build())
    return _NC_CACHE[0]


def run(inputs, trace=False):
    x = np.asarray(inputs["x"], dtype=np.float32)
    Mdev, wl4 = _host_weights(
        np.asarray(inputs["core"]),
        np.asarray(inputs["u0"]), np.asarray(inputs["u1"]),
        np.asarray(inputs["u2"]),
        np.asarray(inputs["a0"]), np.asarray(inputs["a1"]),
        np.asarray(inputs["a2"]),
    )
    xd = _host_x(x)
    nc = _get_nc()
    in_maps = []
    for i in range(NCORES):
        in_maps.append({
            "x": xd[i],
            "m": Mdev,
            "wl": wl4,
        })
    res = run_bass_kernel_spmd(
        nc, in_maps, core_ids=list(range(NCORES)), trace=trace,
    )
    y = np.concatenate([np.asarray(r["y"]) for r in res.results], axis=0)
    y = y.astype(np.float32).reshape(4, 64, 8, 256, 128)
    return y, res


def kernel(**inputs) -> np.ndarray:
    y, _ = run(inputs, trace=False)
    return y
